# revision 1
# baseline (speedup 1.0000x reference)
"""GraphTransformer (4-layer masked dense attention) on 8 TRN2 NeuronCores.

Sharding: nodes (rows of x / rows of adj) split 512/core. Weights replicated.
Per layer each core projects q/kT/v for its own 512 nodes, AllGathers kT
(critical path) and v (overlapped) in bf16, then computes masked softmax
attention + FFN for its rows.

Structural folds (host side):
  * pe[0] into emb bias; 1/sqrt(DH) into qw/qb; v bias into f1 bias.
  * W2 of layer l into the q/k/v weights of layer l+1 and into the output
    projection: the carried activation is zT (relu output), so the FFN's
    second matmul disappears from the device and the next layer's k
    projection starts one pipeline stage earlier.
  * FFN W1 runs on the UNNORMALIZED attention accumulator; the softmax
    denominator (reciprocal + partition broadcast) is applied between W1 and
    relu, off the critical path.

Layouts: see per-tile comments. scoresT is [m, n] so softmax reduction runs
over the partition axis: exp on ACT, 0/1-mask multiply + f32 accumulate on
DVE, final cross-partition sum via a ones-matmul. No max-subtraction
(scores are O(1); masked entries become exp*0, matching exp(-1e9)=0).
"""

import sys

sys.path.insert(0, "/opt/trn_rl_repo")

import numpy as np
import ml_dtypes

from concourse import bass, bacc, tile, mybir, bass_utils
from concourse.bass import _add_dep_helper

N, DIN, DH, DOUT, L = 4096, 512, 512, 256, 4
NCORES = 8
NP_ = N // NCORES          # 512 nodes per core
BF16 = mybir.dt.bfloat16
F32 = mybir.dt.float32
AF = mybir.ActivationFunctionType
FP8 = mybir.dt.float8e4

_cache = {}


def _build():
    nc = bacc.Bacc(trn_type="TRN2", num_devices=NCORES)

    xT_h = nc.dram_tensor("xT", [DIN, NP_], BF16, kind="ExternalInput")
    maskT_h = nc.dram_tensor("maskT", [N, NP_], FP8, kind="ExternalInput")
    qw_h = nc.dram_tensor("qw", [L * DH, DH], BF16, kind="ExternalInput")
    kw_h = nc.dram_tensor("kw", [L * DH, DH], BF16, kind="ExternalInput")
    vw_h = nc.dram_tensor("vw", [L * DH, DH], BF16, kind="ExternalInput")
    f1w_h = nc.dram_tensor("f1w", [L * DH, DH], BF16, kind="ExternalInput")
    qb_h = nc.dram_tensor("qb", [128, 16], F32, kind="ExternalInput")
    kb_h = nc.dram_tensor("kb", [128, 16], F32, kind="ExternalInput")
    f1b_h = nc.dram_tensor("f1b", [128, 16], F32, kind="ExternalInput")
    outw_h = nc.dram_tensor("outw", [DH, DOUT], BF16, kind="ExternalInput")
    outb_h = nc.dram_tensor("outb", [1, DOUT], BF16, kind="ExternalInput")
    out_h = nc.dram_tensor("out", [NP_, DOUT], F32, kind="ExternalOutput")

    with tile.TileContext(nc) as tc:
        with (
            tc.tile_pool(name="cpool", bufs=1) as cpool,
            tc.tile_pool(name="wpool", bufs=2) as wpool,
            tc.tile_pool(name="apool", bufs=1) as apool,
            tc.tile_pool(name="zpool", bufs=2) as zpool,
            tc.tile_pool(name="gpool", bufs=1) as gpool,
            tc.tile_pool(name="upool", bufs=32) as upool,
            tc.tile_pool(name="tpool", bufs=2) as tpool,
            tc.tile_pool(name="osb", bufs=2) as osbpool,
            tc.tile_pool(name="spool", bufs=3, space="PSUM") as spool,
            tc.tile_pool(name="opool", bufs=1, space="PSUM") as opool,
            tc.tile_pool(name="dpool", bufs=1, space="PSUM") as dpool,
            tc.tile_pool(name="dram", bufs=2, space="DRAM") as dram,
        ):
            # ---- inputs needed for the first k projection go first ----
            xT_s = apool.tile([128, 4 * NP_], BF16, name="xT_s", tag="xT")
            for t in range(4):
                nc.sync.dma_start(
                    xT_s[:, t * NP_:(t + 1) * NP_], xT_h[t * 128:(t + 1) * 128, :]
                )

            def load_w(src, l, nm, gate=None):
                w = wpool.tile([128, 4 * DH], BF16, name=f"{nm}{l}", tag=nm)
                for t in range(4):
                    d = nc.sync.dma_start(
                        w[:, t * DH:(t + 1) * DH],
                        src[l * DH + t * 128: l * DH + (t + 1) * 128, :],
                    )
                    if gate is not None:
                        _add_dep_helper(d.ins, gate.ins, sync=True,
                                        reason="weight prefetch after m-loop start")
                return w

            wk = load_w(kw_h, 0, "wk")
            kb_s = cpool.tile([128, 16], F32, name="kb_s")
            nc.sync.dma_start(kb_s[:], kb_h[:, :])

            wq = load_w(qw_h, 0, "wq")
            wv = load_w(vw_h, 0, "wv")
            w1 = load_w(f1w_h, 0, "w1")
            qb_s = cpool.tile([128, 16], F32, name="qb_s")
            nc.sync.dma_start(qb_s[:], qb_h[:, :])
            f1b_s = cpool.tile([128, 16], F32, name="f1b_s")
            nc.sync.dma_start(f1b_s[:], f1b_h[:, :])
            outw_s = cpool.tile([128, 4 * DOUT], BF16, name="outw_s")
            for t in range(4):
                nc.sync.dma_start(
                    outw_s[:, t * DOUT:(t + 1) * DOUT],
                    outw_h[t * 128:(t + 1) * 128, :],
                )
            outb_s = cpool.tile([1, DOUT], BF16, name="outb_s")
            nc.sync.dma_start(outb_s[:], outb_h[:, :])
            ones_col = cpool.tile([128, 1], F32, name="ones_col")
            nc.vector.memset(ones_col[:], 1.0)
            ones1 = cpool.tile([1, 128], BF16, name="ones1")
            nc.vector.memset(ones1[:], 1.0)
            dsum = cpool.tile([128, NP_], F32, name="dsum")
            r_s = cpool.tile([1, NP_], F32, name="r_s")
            R_s = cpool.tile([128, NP_], F32, name="R_s")

            mask_s = cpool.tile([128, 32 * NP_], FP8, name="mask_s")
            zT = None

            # ---- transformer layers ----
            for l in range(L):
                if l > 0:
                    wk = load_w(kw_h, l, "wk", gate=gate)
                    wq = load_w(qw_h, l, "wq", gate=gate)
                    wv = load_w(vw_h, l, "wv", gate=gate)
                    w1 = load_w(f1w_h, l, "w1", gate=gate)
                src = xT_s if l == 0 else zT

                # k projection first: its AllGather is the critical path
                kT_s = apool.tile([128, 4 * NP_], FP8, name=f"kT{l}", tag="kT")
                v_s = apool.tile([128, 4 * NP_], BF16, name=f"v{l}", tag="v")
                qT_s = apool.tile([128, 4 * NP_], FP8, name=f"qT{l}", tag="qT")
                for ec in range(4):
                    ps = spool.tile([128, NP_], F32, name=f"kps{l}_{ec}", tag="ps")
                    for dt in range(4):
                        nc.tensor.matmul(
                            ps[:],
                            lhsT=wk[:, dt * DH + 128 * ec: dt * DH + 128 * ec + 128],
                            rhs=src[:, dt * NP_:(dt + 1) * NP_],
                            start=(dt == 0),
                            stop=(dt == 3),
                        )
                    nc.scalar.activation(
                        kT_s[:, ec * NP_:(ec + 1) * NP_], ps[:], AF.Identity,
                        bias=kb_s[:, l * 4 + ec: l * 4 + ec + 1],
                    )
                agin_k = dram.tile([DH, NP_], FP8, name=f"agink{l}", tag="agink")
                agout_k = dram.tile(
                    [NCORES * DH, NP_], FP8, name=f"agoutk{l}", tag="agoutk",
                    addr_space="Shared",
                )
                last_bounce = None
                for dt in range(4):
                    last_bounce = nc.sync.dma_start(
                        agin_k[dt * 128:(dt + 1) * 128, :],
                        kT_s[:, dt * NP_:(dt + 1) * NP_],
                    )
                nc.gpsimd.collective_compute(
                    "AllGather",
                    mybir.AluOpType.bypass,
                    replica_groups=[list(range(NCORES))],
                    ins=[agin_k[:, :].opt()],
                    outs=[agout_k[:, :].opt()],
                )

                if l == 0:
                    # mask rides out the collectives; the explicit dep keeps
                    # its 32 queue-filling DMAs from starting before the
                    # critical k bounce.
                    for b in range(32):
                        d = nc.sync.dma_start(
                            mask_s[:, b * NP_:(b + 1) * NP_],
                            maskT_h[b * 128:(b + 1) * 128, :],
                        )
                        _add_dep_helper(
                            d.ins, last_bounce.ins, sync=True,
                            reason="mask load after k bounce",
                        )

                # v projection, then its own (overlappable) AllGather
                for nt in range(4):
                    ps = spool.tile([128, NP_], F32, name=f"vps{l}_{nt}", tag="ps")
                    for dt in range(4):
                        nc.tensor.matmul(
                            ps[:],
                            lhsT=src[:, dt * NP_ + 128 * nt: dt * NP_ + 128 * nt + 128],
                            rhs=wv[:, dt * DH:(dt + 1) * DH],
                            start=(dt == 0),
                            stop=(dt == 3),
                        )
                    nc.scalar.copy(v_s[:, nt * NP_:(nt + 1) * NP_], ps[:])
                agin_va = dram.tile([DH // 2, NP_], BF16, name=f"aginva{l}", tag="aginva")
                agin_vb = dram.tile([DH // 2, NP_], BF16, name=f"aginvb{l}", tag="aginvb")
                agout_va = dram.tile(
                    [NCORES * DH // 2, NP_], BF16, name=f"agoutva{l}", tag="agoutva",
                    addr_space="Shared",
                )
                agout_vb = dram.tile(
                    [NCORES * DH // 2, NP_], BF16, name=f"agoutvb{l}", tag="agoutvb",
                    addr_space="Shared",
                )
                for nt in range(4):
                    dst = agin_va if nt < 2 else agin_vb
                    nc.sync.dma_start(
                        dst[(nt % 2) * 128:(nt % 2 + 1) * 128, :],
                        v_s[:, nt * NP_:(nt + 1) * NP_],
                    )
                nc.gpsimd.collective_compute(
                    "AllGather",
                    mybir.AluOpType.bypass,
                    replica_groups=[list(range(NCORES))],
                    ins=[agin_va[:, :].opt()],
                    outs=[agout_va[:, :].opt()],
                )
                nc.gpsimd.collective_compute(
                    "AllGather",
                    mybir.AluOpType.bypass,
                    replica_groups=[list(range(NCORES))],
                    ins=[agin_vb[:, :].opt()],
                    outs=[agout_vb[:, :].opt()],
                )

                # q projection (overlaps the collectives)
                for ec in range(4):
                    ps = spool.tile([128, NP_], F32, name=f"qps{l}_{ec}", tag="ps")
                    for dt in range(4):
                        nc.tensor.matmul(
                            ps[:],
                            lhsT=wq[:, dt * DH + 128 * ec: dt * DH + 128 * ec + 128],
                            rhs=src[:, dt * NP_:(dt + 1) * NP_],
                            start=(dt == 0),
                            stop=(dt == 3),
                        )
                    nc.scalar.activation(
                        qT_s[:, ec * NP_:(ec + 1) * NP_], ps[:], AF.Identity,
                        bias=qb_s[:, l * 4 + ec: l * 4 + ec + 1],
                    )

                # pull gathered K^T / V into SBUF, K first (scores need it);
                # block 0 split 4-way across queues so scores start sooner
                Gk = gpool.tile([128, 32 * NP_], FP8, name=f"Gk{l}", tag="Gk")
                Gv = gpool.tile([128, 32 * NP_], BF16, name=f"Gv{l}", tag="Gv")
                eng = [nc.sync, nc.sync, nc.sync, nc.sync]
                engv = eng
                for c in range(NCORES):
                    for dt in range(4):
                        b = c * 4 + dt
                        eng[dt].dma_start(
                            Gk[:, b * NP_:(b + 1) * NP_],
                            agout_k[c * DH + dt * 128: c * DH + (dt + 1) * 128, :],
                        )
                for half, src_v in ((0, agout_va), (1, agout_vb)):
                    for c in range(NCORES):
                        for dt2 in range(2):
                            dt = half * 2 + dt2
                            b = c * 4 + dt
                            nc.sync.dma_start(
                                Gv[:, b * NP_:(b + 1) * NP_],
                                src_v[c * DH // 2 + dt2 * 128:
                                      c * DH // 2 + (dt2 + 1) * 128, :],
                            )

                # masked attention, scores kept transposed [m, n]
                nc.vector.memset(dsum[:], 0.0)
                o_ps = [
                    opool.tile([128, NP_], F32, name=f"o{l}_{ec}", tag=f"o{ec}")
                    for ec in range(4)
                ]
                for c in range(NCORES):
                    for mt in range(4):
                        b = c * 4 + mt
                        ps = spool.tile([128, NP_], F32, name=f"s{l}_{b}", tag="ps")
                        for dt in range(4):
                            nc.tensor.matmul(
                                ps[:],
                                lhsT=Gk[:, (c * 4 + dt) * NP_ + 128 * mt:
                                        (c * 4 + dt) * NP_ + 128 * mt + 128],
                                rhs=qT_s[:, dt * NP_:(dt + 1) * NP_],
                                start=(dt == 0),
                                stop=(dt == 3),
                            )
                        u = upool.tile([128, NP_], BF16, name=f"u{l}_{b}", tag="u")
                        e_inst = nc.scalar.activation(u[:], ps[:], AF.Exp)
                        if b == 6:
                            gate = e_inst
                        nc.vector.tensor_mul(
                            u[:], u[:], mask_s[:, b * NP_:(b + 1) * NP_]
                        )
                        nc.vector.tensor_add(dsum[:], dsum[:], u[:])
                        for ec in range(4):
                            nc.tensor.matmul(
                                o_ps[ec][:],
                                lhsT=Gv[:, b * NP_ + 128 * ec: b * NP_ + 128 * ec + 128],
                                rhs=u[:],
                                start=(b == 0),
                                stop=(b == 31),
                            )

                # denominator chain, concurrent with W1 below
                den = dpool.tile([1, NP_], F32, name=f"den{l}", tag="den")
                nc.tensor.matmul(den[:], lhsT=ones_col[:], rhs=dsum[:],
                                 start=True, stop=True)
                nc.vector.reciprocal(r_s[:], den[:])
                nc.gpsimd.partition_broadcast(R_s[:], r_s[:])

                # unnormalized attention output straight to SBUF (DVE: the
                # ACT engine is still draining the m-loop exps)
                oU_s = apool.tile([128, 4 * NP_], BF16, name=f"oU{l}", tag="oU")
                for ec in range(4):
                    nc.vector.tensor_copy(
                        oU_s[:, ec * NP_:(ec + 1) * NP_], o_ps[ec][:]
                    )

                # FFN W1 on unnormalized o; normalize + relu afterwards
                zT_new = zpool.tile([128, 4 * NP_], BF16, name=f"zT{l}", tag="zT")
                for fc in range(4):
                    ps = spool.tile([128, NP_], F32, name=f"f1ps{l}_{fc}", tag="ps")
                    for et in range(4):
                        nc.tensor.matmul(
                            ps[:],
                            lhsT=w1[:, et * DH + 128 * fc: et * DH + 128 * fc + 128],
                            rhs=oU_s[:, et * NP_:(et + 1) * NP_],
                            start=(et == 0),
                            stop=(et == 3),
                        )
                    yn = tpool.tile([128, NP_], BF16, name=f"yn{l}_{fc}", tag="yn")
                    nc.vector.tensor_mul(yn[:], ps[:], R_s[:])
                    nc.scalar.activation(
                        zT_new[:, fc * NP_:(fc + 1) * NP_], yn[:], AF.Relu,
                        bias=f1b_s[:, l * 4 + fc: l * 4 + fc + 1],
                    )
                zT = zT_new

            # ---- output projection from zT (W2/out_w folded): [n, dout] ----
            for nt in range(4):
                ps = spool.tile([128, DOUT], F32, name=f"ops{nt}", tag="ps")
                for dt in range(4):
                    nc.tensor.matmul(
                        ps[:],
                        lhsT=zT[:, dt * NP_ + 128 * nt: dt * NP_ + 128 * nt + 128],
                        rhs=outw_s[:, dt * DOUT:(dt + 1) * DOUT],
                        start=(dt == 0),
                        stop=False,
                    )
                nc.tensor.matmul(ps[:], lhsT=ones1[:], rhs=outb_s[:],
                                 start=False, stop=True)
                ob = osbpool.tile([128, DOUT], F32, name=f"ob{nt}", tag="ob")
                nc.scalar.copy(ob[:], ps[:])
                nc.sync.dma_start(out_h[nt * 128:(nt + 1) * 128, :], ob[:])

    nc.compile()
    return nc


def _prepare_in_maps(inputs):
    bf16 = ml_dtypes.bfloat16
    x = np.asarray(inputs["x"], np.float32)
    adj = np.asarray(inputs["adj"])
    emb_w = np.asarray(inputs["emb_w"], np.float32)
    emb_b = np.asarray(inputs["emb_b"], np.float32)
    qw = np.asarray(inputs["qw"], np.float32)
    qb = np.asarray(inputs["qb"], np.float32)
    kw = np.asarray(inputs["kw"], np.float32)
    kb = np.asarray(inputs["kb"], np.float32)
    vw = np.asarray(inputs["vw"], np.float32)
    vb = np.asarray(inputs["vb"], np.float32)
    f1w = np.asarray(inputs["f1w"], np.float32)
    f1b = np.asarray(inputs["f1b"], np.float32)
    f2w = np.asarray(inputs["f2w"], np.float32)
    f2b = np.asarray(inputs["f2b"], np.float32)
    out_w = np.asarray(inputs["out_w"], np.float32)
    out_b = np.asarray(inputs["out_b"], np.float32)

    pe0 = np.zeros(DH, np.float32)
    pe0[1::2] = 1.0
    embb_eff = emb_b + pe0
    scale = np.float32(1.0 / np.sqrt(DH))
    qw_eff = qw * scale
    qb_eff = qb * scale

    # fold W2/b2 of layer l-1 into layer l's projections; carry z instead of h
    qw_z = np.empty_like(qw)
    kw_z = np.empty_like(kw)
    vw_z = np.empty_like(vw)
    qb_z = np.empty_like(qb)
    kb_z = np.empty_like(kb)
    vb_z = np.zeros_like(vb)
    qw_z[0] = emb_w @ qw_eff[0]
    kw_z[0] = emb_w @ kw[0]
    vw_z[0] = emb_w @ vw[0]
    qb_z[0] = embb_eff @ qw_eff[0] + qb_eff[0]
    kb_z[0] = embb_eff @ kw[0] + kb[0]
    vb_z[0] = embb_eff @ vw[0]
    for l in range(1, L):
        qw_z[l] = f2w[l - 1] @ qw_eff[l]
        kw_z[l] = f2w[l - 1] @ kw[l]
        vw_z[l] = f2w[l - 1] @ vw[l]
        qb_z[l] = f2b[l - 1] @ qw_eff[l] + qb_eff[l]
        kb_z[l] = f2b[l - 1] @ kw[l] + kb[l]
        vb_z[l] = f2b[l - 1] @ vw[l]
    f1b_eff = f1b + np.einsum("ld,lde->le", vb + vb_z, f1w)
    outw_z = f2w[L - 1] @ out_w
    outb_z = f2b[L - 1] @ out_w + out_b

    def bias_tile(v):                 # [512] -> [128, 4], col c = v[c*128+p]
        return np.ascontiguousarray(v.reshape(4, 128).T.astype(np.float32))

    def bias16(bl):                   # [L, 512] -> [128, 16], col l*4+c
        return np.ascontiguousarray(
            np.concatenate([bl[l].reshape(4, 128).T for l in range(L)], axis=1)
        ).astype(np.float32)

    def wstack(w):                    # [L, 512, 512] -> [L*512, 512] bf16
        return np.ascontiguousarray(w.reshape(L * DH, DH)).astype(bf16)

    shared = {
        "qw": wstack(qw_z), "kw": wstack(kw_z), "vw": wstack(vw_z),
        "f1w": wstack(f1w),
        "qb": bias16(qb_z), "kb": bias16(kb_z),
        "f1b": bias16(f1b_eff),
        "outw": outw_z.astype(bf16),
        "outb": outb_z.reshape(1, DOUT).astype(bf16),
    }
    in_maps = []
    for c in range(NCORES):
        rows = slice(c * NP_, (c + 1) * NP_)
        m = dict(shared)
        m["xT"] = np.ascontiguousarray(x[rows].T).astype(bf16)
        m["maskT"] = np.ascontiguousarray(
            (adj[rows] > 0).astype(np.float32).T
        ).astype(ml_dtypes.float8_e4m3)
        in_maps.append(m)
    return in_maps


def _run(inputs, trace=False, **kw):
    if "nc" not in _cache:
        _cache["nc"] = _build()
    nc = _cache["nc"]
    in_maps = _prepare_in_maps(inputs)
    res = bass_utils.run_bass_kernel_spmd(
        nc, in_maps, core_ids=list(range(NCORES)), trace=trace, **kw
    )
    out = np.concatenate(
        [np.asarray(res.results[c]["out"], np.float32) for c in range(NCORES)],
        axis=0,
    )[None]
    return out, res


def kernel(**inputs) -> np.ndarray:
    out, _ = _run(inputs, trace=False)
    return out



# revision 9
# speedup vs baseline: 1.1082x; 1.1082x over previous
"""GraphTransformer (4-layer masked dense attention) on 8 TRN2 NeuronCores.

Sharding: nodes (rows of x / rows of adj) split 512/core. Weights replicated.
Per layer each core projects q/kT/v for its own 512 nodes, AllGathers kT
(critical path) and v, then computes masked softmax attention + FFN for its
rows.

Structural folds (host side):
  * pe[0] into emb bias; 1/sqrt(DH) into qw/qb; v bias into f1 bias.
  * W2 of layer l into the q/k/v weights of layer l+1 and into the output
    projection: the carried activation is zT (relu output), so the FFN's
    second matmul disappears from the device and the next layer's k
    projection starts one pipeline stage earlier.
  * FFN W1 runs on the UNNORMALIZED attention accumulator; the softmax
    denominator is applied between W1 and relu, off the critical path.

Perf structure (vs the first working version):
  * Scores matmuls use fp8 DoubleRow perf mode (2 contraction subtiles per
    instruction, 2x throughput). q/kT are fp8 in all layers.
  * Layers 0-2 also run u=exp(s) and v in fp8 so attn@v uses DoubleRow too;
    layer 3 keeps u/v in bf16 (its attention-output noise lands directly in
    the final output; earlier layers' noise is averaged away by subsequent
    attention).
  * m-loop is split into phase A (all scores+exp+mask+dsum) and phase B
    (all o-accumulate matmuls) so o's dependency on the v AllGather can't
    stall the in-order Tensor queue.
  * mask multiplies alternate between DVE and GPSIMD; dsum stays on DVE.
  * A dummy 128B AllGather at kernel start absorbs the one-time collective
    stream barrier (~35us) into the input-load phase.
  * Gathered K/V land in DRAM pre-swizzled (row = p*4+chunk) so each core's
    block pulls into SBUF as one contiguous 256KB DMA.
  * Softmax denominator: all-ones [128,128] matmul broadcasts den across
    partitions; reciprocal runs on ACT (fast) instead of DVE.
"""

import sys

sys.path.insert(0, "/opt/trn_rl_repo")

import numpy as np
import ml_dtypes

from concourse import bass, bacc, tile, mybir, bass_utils
from concourse.bass import _add_dep_helper

N, DIN, DH, DOUT, L = 4096, 512, 512, 256, 4
NCORES = 8
NP_ = N // NCORES          # 512 nodes per core
BF16 = mybir.dt.bfloat16
F32 = mybir.dt.float32
AF = mybir.ActivationFunctionType
FP8 = mybir.dt.float8e4
DR = mybir.MatmulPerfMode.DoubleRow

# layers whose u (exp of scores) and v run in fp8 (DoubleRow attn@v)
FP8_UV = (True, False, False, False)

_cache = {}


def _build():
    nc = bacc.Bacc(trn_type="TRN2", num_devices=NCORES)

    xT_h = nc.dram_tensor("xT", [DIN, NP_], BF16, kind="ExternalInput")
    maskT_h = nc.dram_tensor("maskT", [128, 32 * NP_], FP8, kind="ExternalInput")
    qw_h = nc.dram_tensor("qw", [L * DH, DH], BF16, kind="ExternalInput")
    kw_h = nc.dram_tensor("kw", [L * DH, DH], BF16, kind="ExternalInput")
    vw_h = nc.dram_tensor("vw", [L * DH, DH], BF16, kind="ExternalInput")
    f1w_h = nc.dram_tensor("f1w", [L * DH, DH], BF16, kind="ExternalInput")
    qb_h = nc.dram_tensor("qb", [128, 16], F32, kind="ExternalInput")
    kb_h = nc.dram_tensor("kb", [128, 16], F32, kind="ExternalInput")
    f1b_h = nc.dram_tensor("f1b", [128, 16], F32, kind="ExternalInput")
    outw_h = nc.dram_tensor("outw", [DH, DOUT], BF16, kind="ExternalInput")
    outb_h = nc.dram_tensor("outb", [1, DOUT], BF16, kind="ExternalInput")
    out_h = nc.dram_tensor("out", [NP_, DOUT], F32, kind="ExternalOutput")

    with tile.TileContext(nc) as tc:
        with (
            tc.tile_pool(name="cpool", bufs=1) as cpool,
            tc.tile_pool(name="wpool", bufs=2) as wpool,
            tc.tile_pool(name="apool", bufs=1) as apool,
            tc.tile_pool(name="zpool", bufs=2) as zpool,
            tc.tile_pool(name="gpool", bufs=1) as gpool,
            tc.tile_pool(name="upool", bufs=16) as upool,
            tc.tile_pool(name="tpool", bufs=2) as tpool,
            tc.tile_pool(name="osb", bufs=2) as osbpool,
            tc.tile_pool(name="spool", bufs=3, space="PSUM") as spool,
            tc.tile_pool(name="opool", bufs=1, space="PSUM") as opool,
            tc.tile_pool(name="dpool", bufs=1, space="PSUM") as dpool,
            tc.tile_pool(name="dram", bufs=2, space="DRAM") as dram,
        ):
            # ---- dummy collective: absorbs the one-time CC stream barrier
            # while inputs load and the first k projection runs ----
            dummy_s = cpool.tile([1, 128], BF16, name="dummy_s")
            nc.vector.memset(dummy_s[:], 0.0)
            agin_d = dram.tile([1, 128], BF16, name="agind", tag="agind")
            agout_d = dram.tile(
                [NCORES, 128], BF16, name="agoutd", tag="agoutd",
                addr_space="Shared",
            )
            nc.sync.dma_start(agin_d[:, :], dummy_s[:])
            nc.gpsimd.collective_compute(
                "AllGather",
                mybir.AluOpType.bypass,
                replica_groups=[list(range(NCORES))],
                ins=[agin_d[:, :].opt()],
                outs=[agout_d[:, :].opt()],
            )

            # ---- inputs needed for the first k projection go first ----
            xT_s = apool.tile([128, 4 * NP_], BF16, name="xT_s", tag="xT")
            for t in range(4):
                nc.sync.dma_start(
                    xT_s[:, t * NP_:(t + 1) * NP_], xT_h[t * 128:(t + 1) * 128, :]
                )

            def load_w(src, l, nm, gate=None):
                w = wpool.tile([128, 4 * DH], BF16, name=f"{nm}{l}", tag=nm)
                for t in range(4):
                    d = nc.sync.dma_start(
                        w[:, t * DH:(t + 1) * DH],
                        src[l * DH + t * 128: l * DH + (t + 1) * 128, :],
                    )
                    if gate is not None:
                        _add_dep_helper(d.ins, gate.ins, sync=True,
                                        reason="weight prefetch after m-loop start")
                return w

            wk = load_w(kw_h, 0, "wk")
            kb_s = cpool.tile([128, 16], F32, name="kb_s")
            nc.sync.dma_start(kb_s[:], kb_h[:, :])

            wq = load_w(qw_h, 0, "wq")
            wv = load_w(vw_h, 0, "wv")
            w1 = load_w(f1w_h, 0, "w1")
            qb_s = cpool.tile([128, 16], F32, name="qb_s")
            nc.sync.dma_start(qb_s[:], qb_h[:, :])
            f1b_s = cpool.tile([128, 16], F32, name="f1b_s")
            nc.sync.dma_start(f1b_s[:], f1b_h[:, :])
            outw_s = cpool.tile([128, 4 * DOUT], BF16, name="outw_s")
            for t in range(4):
                nc.sync.dma_start(
                    outw_s[:, t * DOUT:(t + 1) * DOUT],
                    outw_h[t * 128:(t + 1) * 128, :],
                )
            outb_s = cpool.tile([1, DOUT], BF16, name="outb_s")
            nc.sync.dma_start(outb_s[:], outb_h[:, :])
            ones128 = cpool.tile([128, 128], F32, name="ones128")
            nc.vector.memset(ones128[:], 1.0)
            ones1 = cpool.tile([1, 128], BF16, name="ones1")
            nc.vector.memset(ones1[:], 1.0)
            dsum = cpool.tile([128, NP_], F32, name="dsum")
            R_s = cpool.tile([128, NP_], F32, name="R_s")

            # mask blocks, host-reordered: partition p holds mask[m=b*128+p, n]
            mask_s = cpool.tile([128, 32, NP_], FP8, name="mask_s")
            zT = None

            # ---- transformer layers ----
            for l in range(L):
                fp8uv = FP8_UV[l]
                if l > 0:
                    wk = load_w(kw_h, l, "wk", gate=gate)
                    wq = load_w(qw_h, l, "wq", gate=gate)
                    wv = load_w(vw_h, l, "wv", gate=gate)
                    w1 = load_w(f1w_h, l, "w1", gate=gate)
                src = xT_s if l == 0 else zT

                # k projection first: its AllGather is the critical path
                kT_s = apool.tile([128, 4, NP_], FP8, name=f"kT{l}", tag="kT")
                qT_s = apool.tile([128, 4, NP_], FP8, name=f"qT{l}", tag="qT")
                for ec in range(4):
                    ps = spool.tile([128, NP_], F32, name=f"kps{l}_{ec}", tag="ps")
                    for dt in range(4):
                        nc.tensor.matmul(
                            ps[:],
                            lhsT=wk[:, dt * DH + 128 * ec: dt * DH + 128 * ec + 128],
                            rhs=src[:, dt * NP_:(dt + 1) * NP_],
                            start=(dt == 0),
                            stop=(dt == 3),
                        )
                    nc.scalar.activation(
                        kT_s[:, ec:ec + 1, :], ps[:], AF.Identity,
                        bias=kb_s[:, l * 4 + ec: l * 4 + ec + 1],
                    )
                # bounce kT to DRAM pre-swizzled: row p*4+ec so the gathered
                # pull back to SBUF is one contiguous DMA per core
                agin_k = dram.tile([128, 4, NP_], FP8, name=f"agink{l}", tag="agink")
                agout_k = dram.tile(
                    [NCORES, 128, 4, NP_], FP8, name=f"agoutk{l}", tag="agoutk",
                    addr_space="Shared",
                )
                last_bounce = None
                for ec in range(4):
                    last_bounce = nc.sync.dma_start(
                        agin_k[:, ec:ec + 1, :], kT_s[:, ec:ec + 1, :]
                    )
                nc.gpsimd.collective_compute(
                    "AllGather",
                    mybir.AluOpType.bypass,
                    replica_groups=[list(range(NCORES))],
                    ins=[agin_k[:, :, :].opt()],
                    outs=[agout_k[:, :, :, :].opt()],
                )

                if l == 0:
                    # mask rides out the collectives; the explicit dep keeps
                    # its DMAs from starting before the critical k bounce.
                    for t in range(4):
                        d = nc.sync.dma_start(
                            mask_s[:, t * 8:(t + 1) * 8, :],
                            maskT_h[:, t * 8 * NP_:(t + 1) * 8 * NP_],
                        )
                        _add_dep_helper(
                            d.ins, last_bounce.ins, sync=True,
                            reason="mask load after k bounce",
                        )

                # v projection, then its own AllGather (single, fp8 when the
                # layer's attn@v runs in fp8)
                vdt = FP8 if fp8uv else BF16
                v_s = apool.tile([128, 4, DH], vdt, name=f"v{l}", tag=f"v{int(fp8uv)}")
                for nt in range(4):
                    ps = spool.tile([128, NP_], F32, name=f"vps{l}_{nt}", tag="ps")
                    for dt in range(4):
                        nc.tensor.matmul(
                            ps[:],
                            lhsT=src[:, dt * NP_ + 128 * nt: dt * NP_ + 128 * nt + 128],
                            rhs=wv[:, dt * DH:(dt + 1) * DH],
                            start=(dt == 0),
                            stop=(dt == 3),
                        )
                    nc.scalar.copy(v_s[:, nt:nt + 1, :], ps[:])
                if fp8uv:
                    agin_v = dram.tile([128, 4, DH], FP8, name=f"aginv{l}",
                                       tag="aginv8")
                    agout_v = dram.tile(
                        [NCORES, 128, 4, DH], FP8, name=f"agoutv{l}",
                        tag="agoutv8", addr_space="Shared",
                    )
                    for nt in range(4):
                        nc.sync.dma_start(
                            agin_v[:, nt:nt + 1, :], v_s[:, nt:nt + 1, :]
                        )
                    nc.gpsimd.collective_compute(
                        "AllGather",
                        mybir.AluOpType.bypass,
                        replica_groups=[list(range(NCORES))],
                        ins=[agin_v[:, :, :].opt()],
                        outs=[agout_v[:, :, :, :].opt()],
                    )
                    ag_vs = [agout_v]
                else:
                    # bf16 layer: split halves so the first half lands sooner
                    agin_va = dram.tile([128, 2, DH], BF16, name=f"aginva{l}",
                                        tag="aginva")
                    agin_vb = dram.tile([128, 2, DH], BF16, name=f"aginvb{l}",
                                        tag="aginvb")
                    agout_va = dram.tile(
                        [NCORES, 128, 2, DH], BF16, name=f"agoutva{l}",
                        tag="agoutva", addr_space="Shared",
                    )
                    agout_vb = dram.tile(
                        [NCORES, 128, 2, DH], BF16, name=f"agoutvb{l}",
                        tag="agoutvb", addr_space="Shared",
                    )
                    for nt in range(4):
                        dst = agin_va if nt < 2 else agin_vb
                        nc.sync.dma_start(
                            dst[:, nt % 2:nt % 2 + 1, :], v_s[:, nt:nt + 1, :]
                        )
                    for agi, ago in ((agin_va, agout_va), (agin_vb, agout_vb)):
                        nc.gpsimd.collective_compute(
                            "AllGather",
                            mybir.AluOpType.bypass,
                            replica_groups=[list(range(NCORES))],
                            ins=[agi[:, :, :].opt()],
                            outs=[ago[:, :, :, :].opt()],
                        )
                    ag_vs = [agout_va, agout_vb]

                # q projection (overlaps the collectives)
                for ec in range(4):
                    ps = spool.tile([128, NP_], F32, name=f"qps{l}_{ec}", tag="ps")
                    for dt in range(4):
                        nc.tensor.matmul(
                            ps[:],
                            lhsT=wq[:, dt * DH + 128 * ec: dt * DH + 128 * ec + 128],
                            rhs=src[:, dt * NP_:(dt + 1) * NP_],
                            start=(dt == 0),
                            stop=(dt == 3),
                        )
                    nc.scalar.activation(
                        qT_s[:, ec:ec + 1, :], ps[:], AF.Identity,
                        bias=qb_s[:, l * 4 + ec: l * 4 + ec + 1],
                    )

                # pull gathered K^T / V into SBUF; one contiguous DMA per
                # core (the bounce pre-swizzled rows to p*4+chunk)
                Gk = gpool.tile([128, 32, NP_], FP8, name=f"Gk{l}", tag="Gk")
                Gv = gpool.tile([128, 32, DH], vdt, name=f"Gv{l}",
                                tag=f"Gv{int(fp8uv)}")
                for c in range(NCORES):
                    nc.sync.dma_start(
                        Gk[:, c * 4:(c + 1) * 4, :], agout_k[c, :, :, :]
                    )
                if fp8uv:
                    for c in range(NCORES):
                        nc.sync.dma_start(
                            Gv[:, c * 4:(c + 1) * 4, :], agout_v[c, :, :, :]
                        )
                else:
                    for half, src_v in enumerate(ag_vs):
                        for c in range(NCORES):
                            nc.sync.dma_start(
                                Gv[:, c * 4 + 2 * half: c * 4 + 2 * half + 2, :],
                                src_v[c, :, :, :],
                            )

                # ---- phase A: scores (fp8 DoubleRow), exp, mask, dsum ----
                udt = FP8 if fp8uv else BF16
                nc.vector.memset(dsum[:], 0.0)
                u_tiles = []
                for c in range(NCORES):
                    for h in range(2):
                        u_tiles.append(upool.tile(
                            [128, 2, NP_], udt, name=f"u{l}_{c}_{h}",
                            tag=f"u{int(fp8uv)}",
                        ))
                for b in range(32):
                    c, mt = b // 4, b % 4
                    ps = spool.tile([128, NP_], F32, name=f"s{l}_{b}", tag="ps")
                    for dtp in (0, 2):
                        nc.tensor.matmul(
                            ps[:],
                            lhsT=Gk[:, c * 4 + dtp: c * 4 + dtp + 2,
                                    128 * mt: 128 * mt + 128],
                            rhs=qT_s[:, dtp:dtp + 2, :],
                            start=(dtp == 0),
                            stop=(dtp == 2),
                            perf_mode=DR,
                        )
                    ut = u_tiles[c * 2 + mt // 2]
                    j = mt % 2
                    e_inst = nc.scalar.activation(ut[:, j:j + 1, :], ps[:], AF.Exp)
                    if b == 6:
                        gate = e_inst
                    meng = nc.vector if (b % 2 == 0) else nc.gpsimd
                    meng.tensor_mul(
                        ut[:, j:j + 1, :], ut[:, j:j + 1, :],
                        mask_s[:, b:b + 1, :],
                    )
                    nc.vector.tensor_add(dsum[:], dsum[:], ut[:, j:j + 1, :])

                # ---- phase B: o accumulation (DoubleRow when fp8) ----
                o_ps = [
                    opool.tile([128, NP_], F32, name=f"o{l}_{ec}", tag=f"o{ec}")
                    for ec in range(4)
                ]
                if fp8uv:
                    for c in range(NCORES):
                        for h in range(2):
                            ut = u_tiles[c * 2 + h]
                            first = (c == 0 and h == 0)
                            last = (c == NCORES - 1 and h == 1)
                            for ec in range(4):
                                nc.tensor.matmul(
                                    o_ps[ec][:],
                                    lhsT=Gv[:, c * 4 + 2 * h: c * 4 + 2 * h + 2,
                                            128 * ec: 128 * ec + 128],
                                    rhs=ut[:, 0:2, :],
                                    start=first,
                                    stop=last,
                                    perf_mode=DR,
                                )
                else:
                    # half-major order matches the va/vb arrival order
                    for h in range(2):
                        for c in range(NCORES):
                            for j in range(2):
                                ut = u_tiles[c * 2 + h]
                                first = (h == 0 and c == 0 and j == 0)
                                last = (h == 1 and c == NCORES - 1 and j == 1)
                                bb = c * 4 + 2 * h + j
                                for ec in range(4):
                                    nc.tensor.matmul(
                                        o_ps[ec][:],
                                        lhsT=Gv[:, bb:bb + 1,
                                                128 * ec: 128 * ec + 128],
                                        rhs=ut[:, j:j + 1, :],
                                        start=first,
                                        stop=last,
                                    )

                # denominator chain, concurrent with W1 below; the all-ones
                # lhsT broadcasts den across partitions (no partition_broadcast)
                den = dpool.tile([128, NP_], F32, name=f"den{l}", tag="den")
                nc.tensor.matmul(den[:], lhsT=ones128[:], rhs=dsum[:],
                                 start=True, stop=True)
                nc.vector.reciprocal(R_s[:], den[:])

                # unnormalized attention output straight to SBUF
                oU_s = apool.tile([128, 4 * NP_], BF16, name=f"oU{l}", tag="oU")
                for ec in range(4):
                    nc.vector.tensor_copy(
                        oU_s[:, ec * NP_:(ec + 1) * NP_], o_ps[ec][:]
                    )

                # FFN W1 on unnormalized o; normalize + relu afterwards
                zT_new = zpool.tile([128, 4 * NP_], BF16, name=f"zT{l}", tag="zT")
                for fc in range(4):
                    ps = spool.tile([128, NP_], F32, name=f"f1ps{l}_{fc}", tag="ps")
                    for et in range(4):
                        nc.tensor.matmul(
                            ps[:],
                            lhsT=w1[:, et * DH + 128 * fc: et * DH + 128 * fc + 128],
                            rhs=oU_s[:, et * NP_:(et + 1) * NP_],
                            start=(et == 0),
                            stop=(et == 3),
                        )
                    yn = tpool.tile([128, NP_], BF16, name=f"yn{l}_{fc}", tag="yn")
                    nc.vector.tensor_mul(yn[:], ps[:], R_s[:])
                    nc.scalar.activation(
                        zT_new[:, fc * NP_:(fc + 1) * NP_], yn[:], AF.Relu,
                        bias=f1b_s[:, l * 4 + fc: l * 4 + fc + 1],
                    )
                zT = zT_new

            # ---- output projection from zT (W2/out_w folded): [n, dout] ----
            for nt in range(4):
                ps = spool.tile([128, DOUT], F32, name=f"ops{nt}", tag="ps")
                for dt in range(4):
                    nc.tensor.matmul(
                        ps[:],
                        lhsT=zT[:, dt * NP_ + 128 * nt: dt * NP_ + 128 * nt + 128],
                        rhs=outw_s[:, dt * DOUT:(dt + 1) * DOUT],
                        start=(dt == 0),
                        stop=False,
                    )
                nc.tensor.matmul(ps[:], lhsT=ones1[:], rhs=outb_s[:],
                                 start=False, stop=True)
                ob = osbpool.tile([128, DOUT], F32, name=f"ob{nt}", tag="ob")
                nc.scalar.copy(ob[:], ps[:])
                nc.sync.dma_start(out_h[nt * 128:(nt + 1) * 128, :], ob[:])

    nc.compile()
    return nc


def _prepare_in_maps(inputs):
    bf16 = ml_dtypes.bfloat16
    x = np.asarray(inputs["x"], np.float32)
    adj = np.asarray(inputs["adj"])
    emb_w = np.asarray(inputs["emb_w"], np.float32)
    emb_b = np.asarray(inputs["emb_b"], np.float32)
    qw = np.asarray(inputs["qw"], np.float32)
    qb = np.asarray(inputs["qb"], np.float32)
    kw = np.asarray(inputs["kw"], np.float32)
    kb = np.asarray(inputs["kb"], np.float32)
    vw = np.asarray(inputs["vw"], np.float32)
    vb = np.asarray(inputs["vb"], np.float32)
    f1w = np.asarray(inputs["f1w"], np.float32)
    f1b = np.asarray(inputs["f1b"], np.float32)
    f2w = np.asarray(inputs["f2w"], np.float32)
    f2b = np.asarray(inputs["f2b"], np.float32)
    out_w = np.asarray(inputs["out_w"], np.float32)
    out_b = np.asarray(inputs["out_b"], np.float32)

    pe0 = np.zeros(DH, np.float32)
    pe0[1::2] = 1.0
    embb_eff = emb_b + pe0
    scale = np.float32(1.0 / np.sqrt(DH))
    qw_eff = qw * scale
    qb_eff = qb * scale

    # fold W2/b2 of layer l-1 into layer l's projections; carry z instead of h
    qw_z = np.empty_like(qw)
    kw_z = np.empty_like(kw)
    vw_z = np.empty_like(vw)
    qb_z = np.empty_like(qb)
    kb_z = np.empty_like(kb)
    vb_z = np.zeros_like(vb)
    qw_z[0] = emb_w @ qw_eff[0]
    kw_z[0] = emb_w @ kw[0]
    vw_z[0] = emb_w @ vw[0]
    qb_z[0] = embb_eff @ qw_eff[0] + qb_eff[0]
    kb_z[0] = embb_eff @ kw[0] + kb[0]
    vb_z[0] = embb_eff @ vw[0]
    for l in range(1, L):
        qw_z[l] = f2w[l - 1] @ qw_eff[l]
        kw_z[l] = f2w[l - 1] @ kw[l]
        vw_z[l] = f2w[l - 1] @ vw[l]
        qb_z[l] = f2b[l - 1] @ qw_eff[l] + qb_eff[l]
        kb_z[l] = f2b[l - 1] @ kw[l] + kb[l]
        vb_z[l] = f2b[l - 1] @ vw[l]
    f1b_eff = f1b + np.einsum("ld,lde->le", vb + vb_z, f1w)
    outw_z = f2w[L - 1] @ out_w
    outb_z = f2b[L - 1] @ out_w + out_b

    def bias16(bl):                   # [L, 512] -> [128, 16], col l*4+c
        return np.ascontiguousarray(
            np.concatenate([bl[l].reshape(4, 128).T for l in range(L)], axis=1)
        ).astype(np.float32)

    def wstack(w):                    # [L, 512, 512] -> [L*512, 512] bf16
        return np.ascontiguousarray(w.reshape(L * DH, DH)).astype(bf16)

    shared = {
        "qw": wstack(qw_z), "kw": wstack(kw_z), "vw": wstack(vw_z),
        "f1w": wstack(f1w),
        "qb": bias16(qb_z), "kb": bias16(kb_z),
        "f1b": bias16(f1b_eff),
        "outw": outw_z.astype(bf16),
        "outb": outb_z.reshape(1, DOUT).astype(bf16),
    }
    in_maps = []
    for c in range(NCORES):
        rows = slice(c * NP_, (c + 1) * NP_)
        m = dict(shared)
        m["xT"] = np.ascontiguousarray(x[rows].T).astype(bf16)
        # maskT[m, n] for this core's queries n, reordered to [p, b, n] with
        # m = b*128+p, flattened to [128, 32*NP_]
        mT = (adj[rows] > 0).astype(np.float32).T          # [N, NP_]
        mT = mT.reshape(32, 128, NP_).transpose(1, 0, 2)   # [128, 32, NP_]
        m["maskT"] = np.ascontiguousarray(
            mT.reshape(128, 32 * NP_)
        ).astype(ml_dtypes.float8_e4m3)
        in_maps.append(m)
    return in_maps


def _run(inputs, trace=False, **kw):
    if "nc" not in _cache:
        _cache["nc"] = _build()
    nc = _cache["nc"]
    in_maps = _prepare_in_maps(inputs)
    res = bass_utils.run_bass_kernel_spmd(
        nc, in_maps, core_ids=list(range(NCORES)), trace=trace, **kw
    )
    out = np.concatenate(
        [np.asarray(res.results[c]["out"], np.float32) for c in range(NCORES)],
        axis=0,
    )[None]
    return out, res


def kernel(**inputs) -> np.ndarray:
    out, _ = _run(inputs, trace=False)
    return out


# revision 10
# speedup vs baseline: 1.1509x; 1.0385x over previous
"""GraphTransformer (4-layer masked dense attention) on 8 TRN2 NeuronCores.

Sharding: nodes (rows of x / rows of adj) split 512/core. Weights replicated.
Per layer each core projects q/kT/v for its own 512 nodes, AllGathers kT
(critical path) and v, then computes masked softmax attention + FFN for its
rows.

Structural folds (host side):
  * pe[0] into emb bias; 1/sqrt(DH) into qw/qb; v bias into f1 bias.
  * W2 of layer l into the q/k/v weights of layer l+1 and into the output
    projection (carried activation is zT = relu output).
  * k bias dropped entirely: it shifts all scores of a query equally, and
    softmax is invariant to per-query score offsets.
  * FFN W1 runs on the UNNORMALIZED attention accumulator; the softmax
    denominator is applied between W1 and relu, off the critical path.

Perf structure:
  * Scores matmuls use fp8 DoubleRow perf mode (2 contraction subtiles per
    instruction, 2x throughput). q/kT are fp8 in all layers.
  * Layer 0 also runs u=exp(s) and v in fp8 so attn@v uses DoubleRow; for
    layers 1-3 the activations collapse (scores ~1e-3) so exp(s)=1+eps
    would be wiped by fp8's 3-bit mantissa -- they keep u/v in bf16.
  * m-loop is split into phase A (all scores+exp+mask+dsum) and phase B
    (all o-accumulate matmuls) so o's dependency on the v AllGather can't
    stall the in-order Tensor queue.
  * mask multiplies alternate between DVE and GPSIMD; dsum stays on DVE.
  * Next layer's k projection is interleaved into the W1/relu chunk loop
    (reusing the freed o psum banks), so the next kT AllGather issues a
    couple microseconds after zT completes.
  * Gathered K/V land in DRAM pre-swizzled (row = p*4+chunk) so each core's
    block pulls into SBUF as one contiguous 256KB DMA.
  * Softmax denominator: all-ones [128,128] matmul broadcasts den across
    partitions (no partition_broadcast); oU copies run on ACT so DVE's
    reciprocal and the W1 chain don't serialize.
"""

import sys

sys.path.insert(0, "/opt/trn_rl_repo")

import numpy as np
import ml_dtypes

from concourse import bass, bacc, tile, mybir, bass_utils
from concourse.bass import _add_dep_helper

N, DIN, DH, DOUT, L = 4096, 512, 512, 256, 4
NCORES = 8
NP_ = N // NCORES          # 512 nodes per core
BF16 = mybir.dt.bfloat16
F32 = mybir.dt.float32
AF = mybir.ActivationFunctionType
FP8 = mybir.dt.float8e4
DR = mybir.MatmulPerfMode.DoubleRow

# layers whose u (exp of scores) and v run in fp8 (DoubleRow attn@v)
FP8_UV = (True, False, False, False)

_cache = {}


def _build():
    nc = bacc.Bacc(trn_type="TRN2", num_devices=NCORES)

    xT_h = nc.dram_tensor("xT", [DIN, NP_], BF16, kind="ExternalInput")
    maskT_h = nc.dram_tensor("maskT", [128, 32 * NP_], FP8, kind="ExternalInput")
    qw_h = nc.dram_tensor("qw", [L * DH, DH], BF16, kind="ExternalInput")
    kw_h = nc.dram_tensor("kw", [L * DH, DH], BF16, kind="ExternalInput")
    vw_h = nc.dram_tensor("vw", [L * DH, DH], BF16, kind="ExternalInput")
    f1w_h = nc.dram_tensor("f1w", [L * DH, DH], BF16, kind="ExternalInput")
    qb_h = nc.dram_tensor("qb", [128, 16], F32, kind="ExternalInput")
    f1b_h = nc.dram_tensor("f1b", [128, 16], F32, kind="ExternalInput")
    outw_h = nc.dram_tensor("outw", [DH, DOUT], BF16, kind="ExternalInput")
    outb_h = nc.dram_tensor("outb", [1, DOUT], BF16, kind="ExternalInput")
    out_h = nc.dram_tensor("out", [NP_, DOUT], F32, kind="ExternalOutput")

    with tile.TileContext(nc) as tc:
        with (
            tc.tile_pool(name="cpool", bufs=1) as cpool,
            tc.tile_pool(name="wpool", bufs=2) as wpool,
            tc.tile_pool(name="apool", bufs=1) as apool,
            tc.tile_pool(name="zpool", bufs=2) as zpool,
            tc.tile_pool(name="gpool", bufs=1) as gpool,
            tc.tile_pool(name="upool", bufs=16) as upool,
            tc.tile_pool(name="tpool", bufs=2) as tpool,
            tc.tile_pool(name="osb", bufs=2) as osbpool,
            tc.tile_pool(name="spool", bufs=3, space="PSUM") as spool,
            tc.tile_pool(name="opool", bufs=1, space="PSUM") as opool,
            tc.tile_pool(name="dpool", bufs=1, space="PSUM") as dpool,
            tc.tile_pool(name="dram", bufs=2, space="DRAM") as dram,
        ):
            # ---- inputs needed for the first k projection go first ----
            xT_s = apool.tile([128, 4 * NP_], BF16, name="xT_s", tag="xT")
            for t in range(4):
                nc.sync.dma_start(
                    xT_s[:, t * NP_:(t + 1) * NP_], xT_h[t * 128:(t + 1) * 128, :]
                )

            def load_w(src, l, nm, gate=None):
                w = wpool.tile([128, 4 * DH], BF16, name=f"{nm}{l}", tag=nm)
                for t in range(4):
                    d = nc.sync.dma_start(
                        w[:, t * DH:(t + 1) * DH],
                        src[l * DH + t * 128: l * DH + (t + 1) * 128, :],
                    )
                    if gate is not None:
                        _add_dep_helper(d.ins, gate.ins, sync=True,
                                        reason="weight prefetch after m-loop start")
                return w

            wk = load_w(kw_h, 0, "wk")
            wq = load_w(qw_h, 0, "wq")
            wv = load_w(vw_h, 0, "wv")
            w1 = load_w(f1w_h, 0, "w1")
            qb_s = cpool.tile([128, 16], F32, name="qb_s")
            nc.sync.dma_start(qb_s[:], qb_h[:, :])
            f1b_s = cpool.tile([128, 16], F32, name="f1b_s")
            nc.sync.dma_start(f1b_s[:], f1b_h[:, :])
            outw_s = cpool.tile([128, 4 * DOUT], BF16, name="outw_s")
            for t in range(4):
                nc.sync.dma_start(
                    outw_s[:, t * DOUT:(t + 1) * DOUT],
                    outw_h[t * 128:(t + 1) * 128, :],
                )
            outb_s = cpool.tile([1, DOUT], BF16, name="outb_s")
            nc.sync.dma_start(outb_s[:], outb_h[:, :])
            ones128 = cpool.tile([128, 128], F32, name="ones128")
            nc.vector.memset(ones128[:], 1.0)
            ones1 = cpool.tile([1, 128], BF16, name="ones1")
            nc.vector.memset(ones1[:], 1.0)
            dsum = cpool.tile([128, NP_], F32, name="dsum")
            R_s = cpool.tile([128, NP_], F32, name="R_s")

            # mask blocks, host-reordered: partition p holds mask[m=b*128+p, n]
            mask_s = cpool.tile([128, 32, NP_], FP8, name="mask_s")
            zT = None
            kT_s = None

            def kT_tail(l, src_kpj):
                """kT copies + bounce + kAG issue. src_kpj: list of psum tiles
                (from the W1-interleaved projection) or None (layer 0: project
                here from xT)."""
                kT = apool.tile([128, 4, NP_], FP8, name=f"kT{l}", tag="kT")
                for ec in range(4):
                    if src_kpj is None:
                        ps = spool.tile([128, NP_], F32, name=f"kps{l}_{ec}",
                                        tag="ps")
                        for dt in range(4):
                            nc.tensor.matmul(
                                ps[:],
                                lhsT=wk[:, dt * DH + 128 * ec:
                                        dt * DH + 128 * ec + 128],
                                rhs=xT_s[:, dt * NP_:(dt + 1) * NP_],
                                start=(dt == 0),
                                stop=(dt == 3),
                            )
                    else:
                        ps = src_kpj[ec]
                    nc.scalar.copy(kT[:, ec:ec + 1, :], ps[:])
                agin_k = dram.tile([128, 4, NP_], FP8, name=f"agink{l}",
                                   tag="agink")
                agout_k = dram.tile(
                    [NCORES, 128, 4, NP_], FP8, name=f"agoutk{l}", tag="agoutk",
                    addr_space="Shared",
                )
                lastb = None
                for ec in range(4):
                    lastb = nc.sync.dma_start(
                        agin_k[:, ec:ec + 1, :], kT[:, ec:ec + 1, :]
                    )
                nc.gpsimd.collective_compute(
                    "AllGather",
                    mybir.AluOpType.bypass,
                    replica_groups=[list(range(NCORES))],
                    ins=[agin_k[:, :, :].opt()],
                    outs=[agout_k[:, :, :, :].opt()],
                )
                return kT, agout_k, lastb

            # layer 0: project + bounce + gather kT right away
            kT_s, agout_k, last_bounce = kT_tail(0, None)

            # mask rides out the collectives; the explicit dep keeps its DMAs
            # from starting before the critical k bounce.
            for t in range(4):
                d = nc.sync.dma_start(
                    mask_s[:, t * 8:(t + 1) * 8, :],
                    maskT_h[:, t * 8 * NP_:(t + 1) * 8 * NP_],
                )
                _add_dep_helper(d.ins, last_bounce.ins, sync=True,
                                reason="mask load after k bounce")

            # ---- transformer layers ----
            for l in range(L):
                fp8uv = FP8_UV[l]
                if l > 0:
                    wq = load_w(qw_h, l, "wq", gate=gate)
                    wv = load_w(vw_h, l, "wv", gate=gate)
                    w1 = load_w(f1w_h, l, "w1", gate=gate)
                src = xT_s if l == 0 else zT

                # v projection, then its own AllGather (single fp8 when this
                # layer's attn@v runs in fp8, else split bf16 halves)
                vdt = FP8 if fp8uv else BF16
                v_s = apool.tile([128, 4, DH], vdt, name=f"v{l}",
                                 tag=f"v{int(fp8uv)}")
                for nt in range(4):
                    ps = spool.tile([128, NP_], F32, name=f"vps{l}_{nt}", tag="ps")
                    for dt in range(4):
                        nc.tensor.matmul(
                            ps[:],
                            lhsT=src[:, dt * NP_ + 128 * nt: dt * NP_ + 128 * nt + 128],
                            rhs=wv[:, dt * DH:(dt + 1) * DH],
                            start=(dt == 0),
                            stop=(dt == 3),
                        )
                    nc.scalar.copy(v_s[:, nt:nt + 1, :], ps[:])
                if fp8uv:
                    agin_v = dram.tile([128, 4, DH], FP8, name=f"aginv{l}",
                                       tag="aginv8")
                    agout_v = dram.tile(
                        [NCORES, 128, 4, DH], FP8, name=f"agoutv{l}",
                        tag="agoutv8", addr_space="Shared",
                    )
                    for nt in range(4):
                        nc.sync.dma_start(
                            agin_v[:, nt:nt + 1, :], v_s[:, nt:nt + 1, :]
                        )
                    nc.gpsimd.collective_compute(
                        "AllGather",
                        mybir.AluOpType.bypass,
                        replica_groups=[list(range(NCORES))],
                        ins=[agin_v[:, :, :].opt()],
                        outs=[agout_v[:, :, :, :].opt()],
                    )
                    ag_vs = [agout_v]
                else:
                    agin_va = dram.tile([128, 2, DH], BF16, name=f"aginva{l}",
                                        tag="aginva")
                    agin_vb = dram.tile([128, 2, DH], BF16, name=f"aginvb{l}",
                                        tag="aginvb")
                    agout_va = dram.tile(
                        [NCORES, 128, 2, DH], BF16, name=f"agoutva{l}",
                        tag="agoutva", addr_space="Shared",
                    )
                    agout_vb = dram.tile(
                        [NCORES, 128, 2, DH], BF16, name=f"agoutvb{l}",
                        tag="agoutvb", addr_space="Shared",
                    )
                    for nt in range(4):
                        dst = agin_va if nt < 2 else agin_vb
                        nc.sync.dma_start(
                            dst[:, nt % 2:nt % 2 + 1, :], v_s[:, nt:nt + 1, :]
                        )
                    for agi, ago in ((agin_va, agout_va), (agin_vb, agout_vb)):
                        nc.gpsimd.collective_compute(
                            "AllGather",
                            mybir.AluOpType.bypass,
                            replica_groups=[list(range(NCORES))],
                            ins=[agi[:, :, :].opt()],
                            outs=[ago[:, :, :, :].opt()],
                        )
                    ag_vs = [agout_va, agout_vb]

                # q projection (overlaps the collectives)
                qT_s = apool.tile([128, 4, NP_], FP8, name=f"qT{l}", tag="qT")
                for ec in range(4):
                    ps = spool.tile([128, NP_], F32, name=f"qps{l}_{ec}", tag="ps")
                    for dt in range(4):
                        nc.tensor.matmul(
                            ps[:],
                            lhsT=wq[:, dt * DH + 128 * ec: dt * DH + 128 * ec + 128],
                            rhs=src[:, dt * NP_:(dt + 1) * NP_],
                            start=(dt == 0),
                            stop=(dt == 3),
                        )
                    nc.scalar.activation(
                        qT_s[:, ec:ec + 1, :], ps[:], AF.Identity,
                        bias=qb_s[:, l * 4 + ec: l * 4 + ec + 1],
                    )

                # pull gathered K^T / V into SBUF; one contiguous DMA per core
                Gk = gpool.tile([128, 32, NP_], FP8, name=f"Gk{l}", tag="Gk")
                Gv = gpool.tile([128, 32, DH], vdt, name=f"Gv{l}",
                                tag=f"Gv{int(fp8uv)}")
                for c in range(NCORES):
                    nc.sync.dma_start(
                        Gk[:, c * 4:(c + 1) * 4, :], agout_k[c, :, :, :]
                    )
                if fp8uv:
                    for c in range(NCORES):
                        nc.sync.dma_start(
                            Gv[:, c * 4:(c + 1) * 4, :], agout_v[c, :, :, :]
                        )
                else:
                    for half, src_v in enumerate(ag_vs):
                        eng = nc.sync if half == 0 else nc.scalar
                        for c in range(NCORES):
                            eng.dma_start(
                                Gv[:, c * 4 + 2 * half: c * 4 + 2 * half + 2, :],
                                src_v[c, :, :, :],
                            )

                # ---- phase A: scores (fp8 DoubleRow), exp, mask, dsum ----
                udt = FP8 if fp8uv else BF16
                nc.vector.memset(dsum[:], 0.0)
                u_tiles = []
                for c in range(NCORES):
                    for h in range(2):
                        u_tiles.append(upool.tile(
                            [128, 2, NP_], udt, name=f"u{l}_{c}_{h}",
                            tag=f"u{int(fp8uv)}",
                        ))
                for b in range(32):
                    c, mt = b // 4, b % 4
                    ps = spool.tile([128, NP_], F32, name=f"s{l}_{b}", tag="ps")
                    for dtp in (0, 2):
                        nc.tensor.matmul(
                            ps[:],
                            lhsT=Gk[:, c * 4 + dtp: c * 4 + dtp + 2,
                                    128 * mt: 128 * mt + 128],
                            rhs=qT_s[:, dtp:dtp + 2, :],
                            start=(dtp == 0),
                            stop=(dtp == 2),
                            perf_mode=DR,
                        )
                    ut = u_tiles[c * 2 + mt // 2]
                    j = mt % 2
                    e_inst = nc.scalar.activation(ut[:, j:j + 1, :], ps[:], AF.Exp)
                    if b == 6:
                        gate = e_inst
                    meng = nc.vector if (b % 2 == 0) else nc.gpsimd
                    meng.tensor_mul(
                        ut[:, j:j + 1, :], ut[:, j:j + 1, :],
                        mask_s[:, b:b + 1, :],
                    )
                    nc.vector.tensor_add(dsum[:], dsum[:], ut[:, j:j + 1, :])

                # next layer's wk: prefetch now (gate just became available)
                if l < L - 1:
                    wk = load_w(kw_h, l + 1, "wk", gate=gate)

                # ---- phase B: o accumulation (DoubleRow when fp8) ----
                o_ps = [
                    opool.tile([128, NP_], F32, name=f"o{l}_{ec}", tag=f"o{ec}")
                    for ec in range(4)
                ]
                den = dpool.tile([128, NP_], F32, name=f"den{l}", tag="den")

                def den_chain():
                    nc.tensor.matmul(den[:], lhsT=ones128[:], rhs=dsum[:],
                                     start=True, stop=True)
                    nc.vector.reciprocal(R_s[:], den[:])

                if fp8uv:
                    for c in range(NCORES):
                        for h in range(2):
                            ut = u_tiles[c * 2 + h]
                            first = (c == 0 and h == 0)
                            last = (c == NCORES - 1 and h == 1)
                            for ec in range(4):
                                nc.tensor.matmul(
                                    o_ps[ec][:],
                                    lhsT=Gv[:, c * 4 + 2 * h: c * 4 + 2 * h + 2,
                                            128 * ec: 128 * ec + 128],
                                    rhs=ut[:, 0:2, :],
                                    start=first,
                                    stop=last,
                                    perf_mode=DR,
                                )
                    den_chain()
                else:
                    # half-major order matches the va/vb arrival order
                    for h in range(2):
                        for c in range(NCORES):
                            for j in range(2):
                                ut = u_tiles[c * 2 + h]
                                first = (h == 0 and c == 0 and j == 0)
                                last = (h == 1 and c == NCORES - 1 and j == 1)
                                bb = c * 4 + 2 * h + j
                                for ec in range(4):
                                    nc.tensor.matmul(
                                        o_ps[ec][:],
                                        lhsT=Gv[:, bb:bb + 1,
                                                128 * ec: 128 * ec + 128],
                                        rhs=ut[:, j:j + 1, :],
                                        start=first,
                                        stop=last,
                                    )
                        if h == 0:
                            den_chain()

                # unnormalized attention output -> SBUF on ACT (keeps DVE free
                # for the reciprocal / yn chain)
                oU_s = apool.tile([128, 4 * NP_], BF16, name=f"oU{l}", tag="oU")
                for ec in range(4):
                    nc.scalar.copy(oU_s[:, ec * NP_:(ec + 1) * NP_], o_ps[ec][:])

                # FFN W1 on unnormalized o; normalize + relu afterwards.
                # Next layer's k projection interleaves here, accumulating in
                # the freed o psum banks.
                zT_new = zpool.tile([128, 4 * NP_], BF16, name=f"zT{l}", tag="zT")
                kpj = None
                if l < L - 1:
                    kpj = [
                        opool.tile([128, NP_], F32, name=f"kpj{l}_{ec}",
                                   tag=f"o{ec}")
                        for ec in range(4)
                    ]
                for fc in range(4):
                    ps = spool.tile([128, NP_], F32, name=f"f1ps{l}_{fc}", tag="ps")
                    for et in range(4):
                        nc.tensor.matmul(
                            ps[:],
                            lhsT=w1[:, et * DH + 128 * fc: et * DH + 128 * fc + 128],
                            rhs=oU_s[:, et * NP_:(et + 1) * NP_],
                            start=(et == 0),
                            stop=(et == 3),
                        )
                    yn = tpool.tile([128, NP_], BF16, name=f"yn{l}_{fc}", tag="yn")
                    nc.vector.tensor_mul(yn[:], ps[:], R_s[:])
                    nc.scalar.activation(
                        zT_new[:, fc * NP_:(fc + 1) * NP_], yn[:], AF.Relu,
                        bias=f1b_s[:, l * 4 + fc: l * 4 + fc + 1],
                    )
                    if kpj is not None:
                        for ec in range(4):
                            nc.tensor.matmul(
                                kpj[ec][:],
                                lhsT=wk[:, fc * DH + 128 * ec:
                                        fc * DH + 128 * ec + 128],
                                rhs=zT_new[:, fc * NP_:(fc + 1) * NP_],
                                start=(fc == 0),
                                stop=(fc == 3),
                            )
                zT = zT_new
                if kpj is not None:
                    kT_s, agout_k, last_bounce = kT_tail(l + 1, kpj)

            # ---- output projection from zT (W2/out_w folded): [n, dout] ----
            for nt in range(4):
                ps = spool.tile([128, DOUT], F32, name=f"ops{nt}", tag="ps")
                for dt in range(4):
                    nc.tensor.matmul(
                        ps[:],
                        lhsT=zT[:, dt * NP_ + 128 * nt: dt * NP_ + 128 * nt + 128],
                        rhs=outw_s[:, dt * DOUT:(dt + 1) * DOUT],
                        start=(dt == 0),
                        stop=False,
                    )
                nc.tensor.matmul(ps[:], lhsT=ones1[:], rhs=outb_s[:],
                                 start=False, stop=True)
                ob = osbpool.tile([128, DOUT], F32, name=f"ob{nt}", tag="ob")
                nc.scalar.copy(ob[:], ps[:])
                nc.sync.dma_start(out_h[nt * 128:(nt + 1) * 128, :], ob[:])

    nc.compile()
    return nc


def _prepare_in_maps(inputs):
    bf16 = ml_dtypes.bfloat16
    x = np.asarray(inputs["x"], np.float32)
    adj = np.asarray(inputs["adj"])
    emb_w = np.asarray(inputs["emb_w"], np.float32)
    emb_b = np.asarray(inputs["emb_b"], np.float32)
    qw = np.asarray(inputs["qw"], np.float32)
    qb = np.asarray(inputs["qb"], np.float32)
    kw = np.asarray(inputs["kw"], np.float32)
    vw = np.asarray(inputs["vw"], np.float32)
    vb = np.asarray(inputs["vb"], np.float32)
    f1w = np.asarray(inputs["f1w"], np.float32)
    f1b = np.asarray(inputs["f1b"], np.float32)
    f2w = np.asarray(inputs["f2w"], np.float32)
    f2b = np.asarray(inputs["f2b"], np.float32)
    out_w = np.asarray(inputs["out_w"], np.float32)
    out_b = np.asarray(inputs["out_b"], np.float32)

    pe0 = np.zeros(DH, np.float32)
    pe0[1::2] = 1.0
    embb_eff = emb_b + pe0
    scale = np.float32(1.0 / np.sqrt(DH))
    qw_eff = qw * scale
    qb_eff = qb * scale

    # fold W2/b2 of layer l-1 into layer l's projections; carry z instead of h
    qw_z = np.empty_like(qw)
    kw_z = np.empty_like(kw)
    vw_z = np.empty_like(vw)
    qb_z = np.empty_like(qb)
    vb_z = np.zeros_like(vb)
    qw_z[0] = emb_w @ qw_eff[0]
    kw_z[0] = emb_w @ kw[0]
    vw_z[0] = emb_w @ vw[0]
    qb_z[0] = embb_eff @ qw_eff[0] + qb_eff[0]
    vb_z[0] = embb_eff @ vw[0]
    for l in range(1, L):
        qw_z[l] = f2w[l - 1] @ qw_eff[l]
        kw_z[l] = f2w[l - 1] @ kw[l]
        vw_z[l] = f2w[l - 1] @ vw[l]
        qb_z[l] = f2b[l - 1] @ qw_eff[l] + qb_eff[l]
        vb_z[l] = f2b[l - 1] @ vw[l]
    f1b_eff = f1b + np.einsum("ld,lde->le", vb + vb_z, f1w)
    outw_z = f2w[L - 1] @ out_w
    outb_z = f2b[L - 1] @ out_w + out_b

    def bias16(bl):                   # [L, 512] -> [128, 16], col l*4+c
        return np.ascontiguousarray(
            np.concatenate([bl[l].reshape(4, 128).T for l in range(L)], axis=1)
        ).astype(np.float32)

    def wstack(w):                    # [L, 512, 512] -> [L*512, 512] bf16
        return np.ascontiguousarray(w.reshape(L * DH, DH)).astype(bf16)

    shared = {
        "qw": wstack(qw_z), "kw": wstack(kw_z), "vw": wstack(vw_z),
        "f1w": wstack(f1w),
        "qb": bias16(qb_z),
        "f1b": bias16(f1b_eff),
        "outw": outw_z.astype(bf16),
        "outb": outb_z.reshape(1, DOUT).astype(bf16),
    }
    in_maps = []
    for c in range(NCORES):
        rows = slice(c * NP_, (c + 1) * NP_)
        m = dict(shared)
        m["xT"] = np.ascontiguousarray(x[rows].T).astype(bf16)
        # maskT[m, n] for this core's queries n, reordered to [p, b, n] with
        # m = b*128+p, flattened to [128, 32*NP_]
        mT = (adj[rows] > 0).astype(np.float32).T          # [N, NP_]
        mT = mT.reshape(32, 128, NP_).transpose(1, 0, 2)   # [128, 32, NP_]
        m["maskT"] = np.ascontiguousarray(
            mT.reshape(128, 32 * NP_)
        ).astype(ml_dtypes.float8_e4m3)
        in_maps.append(m)
    return in_maps


def _run(inputs, trace=False, **kw):
    if "nc" not in _cache:
        _cache["nc"] = _build()
    nc = _cache["nc"]
    in_maps = _prepare_in_maps(inputs)
    res = bass_utils.run_bass_kernel_spmd(
        nc, in_maps, core_ids=list(range(NCORES)), trace=trace, **kw
    )
    out = np.concatenate(
        [np.asarray(res.results[c]["out"], np.float32) for c in range(NCORES)],
        axis=0,
    )[None]
    return out, res


def kernel(**inputs) -> np.ndarray:
    out, _ = _run(inputs, trace=False)
    return out
